# revision 27
# baseline (speedup 1.0000x reference)
"""AttentionPool kernel for Trainium2, 8 NeuronCores (SPMD data-parallel).

Reference computation (per graph g with atoms A_g, uniform |A_g| = 32):
    h = X @ W.T                              [131072, 512]
    s = leakyrelu(sum(att * h, -1), 0.2)     [131072]
    w = segment_softmax(s)                   per graph
    out[g] = sum_{a in A_g} w[a] * h[a]      [4096, 512]

Algebraic refactor (pool-first; avoids the 69-GFLOP h matmul AND any
transpose of X):
    v  = W.T @ att  (host input prep, tiny)
    s  = lrelu(X @ v)        fused per-tile dot product (stt+accum), split
                             across DVE and GpSimd (both run the same stt;
                             GpSimd at ~0.6 efficiency takes 3 of 8 tiles
                             per batch so neither engine is the bottleneck)
    e  = exp(s)              no max-subtraction needed (|s| <~ 8); lrelu+exp
                             as two tiny [128,8] ACT ops per batch
    em = e * mask01          bf16 masked stationary for the pool matmul,
                             built by ONE DVE broadcast-multiply per batch
                             (replaces 8 expensive per-tile ACT exps)
    P[g] = sum_{a in A_g} e[a] X[a]   PE matmul per 128-atom tile with the
                             [128,32] masked-e stationary; 8-tile batches
                             accumulate a 32-aligned PSUM partition window
    d[g] = ONE tiny matmul per batch: lhsT = emq [128,(8k,4q)] (e masked by
                             quarter, columns already in graph order), rhs =
                             ones -> [32,1] PSUM window (replaces 8 per-tile
                             den matmuls -> big PE queue savings)
    out = (P @ W.T) / d      per-core projection (PE transposes + matmul);
                             1/d rides the final PSUM->SBUF copy so the
                             projection chain never waits on the reciprocal

All heavy data in bf16 (X converted host-side -> 17 MB DMA per core,
matmuls at 1 cyc/row); s/den/PSUM accumulate in fp32. Rel err ~4e-3 vs
the 2e-2 gate. X streams as 512 KiB DMA chunks; group tails are
software-pipelined one batch into the next group, and a small final group
+ split lrelu keep the exposed tail short.

Sharding: 8 cores x 16384 atoms (= 512 graphs, graph-aligned). W/att
replicated. Output slices concatenated on host. Non-uniform segment sizes
fall back to an exact numpy path (never triggered by the fixed harness
inputs, which are uniform 32 atoms/graph).
"""

import numpy as np

N_ATOMS = 131072
FEAT = 512
N_GRAPHS = 4096
NEG_SLOPE = 0.2
N_CORES = 8

P = 128                      # partitions / atoms per tile
NA_CORE = N_ATOMS // N_CORES         # 16384 atoms per core
NT = NA_CORE // P                    # 128 tiles per core
NG_CORE = N_GRAPHS // N_CORES        # 512 graphs per core
GPT = P // 32                        # 4 graphs per tile (uniform 32 atoms/graph)
TPG = P // GPT                       # 32 tiles per 128-graph group
NGRP = NT // TPG                     # 4 groups of 128 graphs per core
FCH = FEAT // P                      # 4 feature chunks
DMA_GRP = 4                          # X tiles per input DMA (512 KiB in bf16)
W32 = 8 * GPT                        # stationary width = graphs per 8-tile batch
TT_K = (1, 3, 5, 6)                  # tiles/batch scored via DVE-tt + ACT-accum
WARM_V = 17                          # v-rhs warm-up matmuls (512-stream)
_CACHED = {}


def _build_program():
    import concourse.bacc as bacc
    import concourse.mybir as mybir
    import concourse.tile as tile
    from concourse.masks import make_identity
    from contextlib import ExitStack

    F32 = mybir.dt.float32
    BF16 = mybir.dt.bfloat16
    FP8 = mybir.dt.float8e4
    MULT = mybir.AluOpType.mult
    EXP = mybir.ActivationFunctionType.Exp
    MAX = mybir.AluOpType.max
    COPY = mybir.ActivationFunctionType.Copy

    nc = bacc.Bacc("TRN2", target_bir_lowering=False, debug=False,
                   num_devices=N_CORES)

    x = nc.dram_tensor("x", [NA_CORE, FEAT], BF16, kind="ExternalInput").ap()
    wt = nc.dram_tensor("wt", [FEAT, FEAT], BF16, kind="ExternalInput").ap()
    vrep = nc.dram_tensor("vrep", [P, FEAT], BF16, kind="ExternalInput").ap()
    mask01 = nc.dram_tensor("mask01", [P, 8, W32], BF16,
                            kind="ExternalInput").ap()
    qmask = nc.dram_tensor("qmask", [P, GPT], BF16, kind="ExternalInput").ap()
    out = nc.dram_tensor("out", [NG_CORE, FEAT], F32, kind="ExternalOutput").ap()

    x_r8 = x.rearrange("(n o p) f -> n p o f", o=DMA_GRP, p=P)
    x_t = x.rearrange("(t p) f -> t p f", p=P)

    with tile.TileContext(nc) as tc, ExitStack() as ctx:
        singles = ctx.enter_context(tc.tile_pool(name="singles", bufs=1))
        xpool = ctx.enter_context(tc.tile_pool(name="xpool", bufs=12))
        x4pool = ctx.enter_context(tc.tile_pool(name="x4pool", bufs=2))
        spool = ctx.enter_context(tc.tile_pool(name="spool", bufs=16))
        junkp = ctx.enter_context(tc.tile_pool(name="junkp", bufs=14))
        gjunkp = ctx.enter_context(tc.tile_pool(name="gjunkp", bufs=6))
        actjp = ctx.enter_context(tc.tile_pool(name="actjp", bufs=6))
        empool = ctx.enter_context(tc.tile_pool(name="empool", bufs=2))
        eqpool = ctx.enter_context(tc.tile_pool(name="eqpool", bufs=4))
        smallp = ctx.enter_context(tc.tile_pool(name="smallp", bufs=4))
        pooledp = ctx.enter_context(tc.tile_pool(name="pooledp", bufs=2))
        ptp = ctx.enter_context(tc.tile_pool(name="ptp", bufs=4))
        outp = ctx.enter_context(tc.tile_pool(name="outp", bufs=2))
        ps_pool = ctx.enter_context(tc.tile_pool(name="ps_pool", bufs=2, space="PSUM"))
        ps_den = ctx.enter_context(tc.tile_pool(name="ps_den", bufs=2, space="PSUM"))
        ps_misc = ctx.enter_context(tc.tile_pool(name="ps_misc", bufs=2, space="PSUM"))
        ps_out = ctx.enter_context(tc.tile_pool(name="ps_out", bufs=2, space="PSUM"))

        # ---- v + a tiny first X chunk (fast first score), then constants,
        # then the streaming 8-tile chunks ----
        first_chunks = []                      # tiles 0..7 in 1/1/2/4 chunks
        v_rep = singles.tile([P, FEAT], BF16)
        nc.sync.dma_start(out=v_rep, in_=vrep)
        fc0 = x4pool.tile([P, 1, FEAT], BF16, tag="fc0", name="fc0")
        nc.sync.dma_start(out=fc0, in_=x_t[0:1].rearrange("o p f -> p o f"))
        first_chunks.append((0, 1, fc0))
        for ci, (t0, sz) in enumerate([(1, 1), (2, 2), (4, 4)], start=1):
            fc = x4pool.tile([P, sz, FEAT], BF16, tag=f"fc{ci}",
                             name=f"fc{ci}")
            nc.sync.dma_start(out=fc, in_=x_t[t0:t0 + sz].rearrange(
                "o p f -> p o f"))
            first_chunks.append((t0, sz, fc))
        mk_sb = singles.tile([P, 8, W32], BF16)
        nc.sync.dma_start(out=mk_sb, in_=mask01)
        qm_sb = singles.tile([P, GPT], BF16)
        nc.sync.dma_start(out=qm_sb, in_=qmask)
        chunks = {}

        def issue_chunk(n):
            ch = xpool.tile([P, DMA_GRP, FEAT], BF16, tag="x8", name=f"x8_{n}")
            nc.sync.dma_start(out=ch, in_=x_r8[n])
            chunks[n] = ch

        for _n in (2, 3, 4, 5, 6):
            issue_chunk(_n)
        wt_sb = singles.tile([P, FCH, FEAT], BF16)
        nc.sync.dma_start(out=wt_sb, in_=wt.rearrange("(c p) f -> p c f", p=P))
        ident = singles.tile([P, P], BF16)
        make_identity(nc, ident)
        ones_col = singles.tile([P, 1], BF16)
        nc.vector.memset(ones_col, 1.0)
        # preload the ACT exp table during the fill (first real exp would
        # otherwise eat a 1.3us ACT_TABLE_LOAD on the critical path)
        exp_warm = spool.tile([P, 1], F32, tag="s_b", name="exp_warm")
        nc.vector.memset(exp_warm, 0.0)
        exp_warm2 = spool.tile([P, 1], F32, tag="s_lr", name="exp_warm2")
        nc.scalar.activation(out=exp_warm2, in_=exp_warm, func=EXP)

        # warm up the PE (HAM clock gate) while the score pipeline fills
        warm_ps = ps_out.tile([P, FEAT], F32, tag="ops", name="warm_ps")
        for wi in range(WARM_V):
            nc.tensor.matmul(warm_ps, lhsT=ident, rhs=v_rep,
                             start=(wi == 0), stop=(wi == WARM_V - 1))

        def xref(t):
            """SBUF AP for tile t; issues the owning chunk DMA at boundaries."""
            if t < 8:
                for t0, sz, fc in first_chunks:
                    if t0 <= t < t0 + sz:
                        return fc[:, t - t0, :]
            n, o = divmod(t, DMA_GRP)
            if n not in chunks:
                issue_chunk(n)
            return chunks[n][:, o, :]

        def emit_batch(t0, bu, pool_ps, den_ps, em_g, splits=(0,),
                       offload=True, extra_off=False):
            s_b = spool.tile([P, 8], F32, tag="s_b")
            s_lr = spool.tile([P, 8], F32, tag="s_lr")
            e_b = spool.tile([P, 8], F32, tag="e_b")
            emq = eqpool.tile([P, 8, GPT], BF16, tag="emq")
            xts = [xref(t0 + bu * 8 + k) for k in range(8)]
            em_b = em_g[:, bu * 8:(bu + 1) * 8, :]
            win = pool_ps[bu * W32:(bu + 1) * W32, :]
            dwin = den_ps[bu * W32:(bu + 1) * W32, :]
            # offloaded pairs: ONE 2x-speed DVE multiply covers 2 tiles
            # (strided view into the chunk tile), then 2 ACT accum-sums
            jt2 = {}
            if offload:
                base = t0 + bu * 8
                n0 = base // DMA_GRP
                for pi, (nn, sl, ks) in enumerate(
                        [(n0, slice(1, 4, 2), (1, 3)),
                         (n0 + 1, slice(1, 3), (5, 6))]):
                    jt = gjunkp.tile([P, 2, FEAT], BF16, tag="gjunk")
                    jt2[ks] = jt
            # the final batch runs per pair of tiles so the exposed tail
            # is only ~2 tiles deep
            for h in splits:
                hw_ = 8 // len(splits)
                for k in range(h, h + hw_):
                    if offload and k in (1, 5):
                        # paired product for (k, k') on DVE at 2x
                        ks = (1, 3) if k == 1 else (5, 6)
                        nn = n0 + (0 if k == 1 else 1)
                        sl = slice(1, 4, 2) if k == 1 else slice(1, 3)
                        nc.vector.tensor_tensor(
                            out=jt2[ks], in0=chunks[nn][:, sl, :],
                            in1=v_rep.unsqueeze(1).broadcast_to(
                                [P, 2, FEAT]),
                            op=MULT)
                    if k in TT_K and offload:
                        ks = (1, 3) if k in (1, 3) else (5, 6)
                        ja = actjp.tile([P, FEAT], FP8, tag="actj")
                        nc.scalar.activation(
                            out=ja, in_=jt2[ks][:, ks.index(k), :],
                            func=COPY, accum_out=s_b[:, k:k + 1])
                    elif k == 2 and offload and extra_off:
                        # odd batches shift one more dot to ACT (z=5)
                        jt1 = gjunkp.tile([P, 2, FEAT], BF16, tag="gjunk")
                        nc.vector.tensor_tensor(
                            out=jt1[:, 0, :], in0=xts[k], in1=v_rep,
                            op=MULT)
                        ja = actjp.tile([P, FEAT], FP8, tag="actj")
                        nc.scalar.activation(
                            out=ja, in_=jt1[:, 0, :],
                            func=COPY, accum_out=s_b[:, k:k + 1])
                    else:
                        junk = junkp.tile([P, FEAT], FP8, tag="junk")
                        nc.vector.scalar_tensor_tensor(
                            out=junk, in0=xts[k], scalar=1.0, in1=v_rep,
                            op0=MULT, op1=MULT, accum_out=s_b[:, k:k + 1])
                # lrelu on DVE (the ACT Lrelu ignores alpha: hardwired 0.01)
                nc.vector.scalar_tensor_tensor(
                    out=s_lr[:, h:h + hw_], in0=s_b[:, h:h + hw_],
                    scalar=NEG_SLOPE, in1=s_b[:, h:h + hw_],
                    op0=MULT, op1=MAX)
                nc.scalar.activation(out=e_b[:, h:h + hw_],
                                     in_=s_lr[:, h:h + hw_], func=EXP)
                nc.gpsimd.tensor_tensor(
                    out=em_b[:, h:h + hw_, :], in0=mk_sb[:, h:h + hw_, :],
                    in1=e_b[:, h:h + hw_].unsqueeze(2).broadcast_to(
                        [P, hw_, W32]),
                    op=MULT)
                for k in range(h, h + hw_):
                    nc.tensor.matmul(win, lhsT=em_b[:, k, :], rhs=xts[k],
                                     start=(k == 0), stop=(k == 7),
                                     tile_position=(0, bu * W32))
            # den for the whole batch in ONE tiny matmul: emq columns are
            # (k,q) flattened = graph order within the batch
            nc.gpsimd.tensor_tensor(
                out=emq,
                in0=qm_sb.unsqueeze(1).broadcast_to([P, 8, GPT]),
                in1=e_b.unsqueeze(2).broadcast_to([P, 8, GPT]),
                op=MULT)
            nc.tensor.matmul(dwin, lhsT=emq, rhs=ones_col,
                             start=True, stop=True,
                             tile_position=(0, bu * W32))

        def emit_tail(goff, nb, pool_ps, den_ps):
            # ---- projection: out[goff:goff+NP] = (pool @ W.T) / den; the
            # 1/den scale rides on the final PSUM->SBUF copy so the
            # transpose/matmul chain never waits on the reciprocal ----
            NP = nb * W32
            denr = smallp.tile([P, 1], F32, tag="denr")
            nc.vector.reciprocal(denr[:NP, :], den_ps[:NP, :])
            pooled = pooledp.tile([P, FEAT], BF16, tag="pooled")
            out_ps = ps_out.tile([P, FEAT], F32, tag="ops")
            # one big PSUM->SBUF copy (vs 4 chunk copies: ACT overhead)
            nc.scalar.copy(pooled[:NP, :], pool_ps[:NP, :])
            tr_ps = ps_misc.tile([P, FCH, P], BF16, tag="tr", name="tr_ps")
            pt = ptp.tile([P, FCH, P], BF16, tag="pt")
            for c in range(FCH):
                nc.tensor.transpose(tr_ps[:, c, :NP],
                                    pooled[:NP, c * P:(c + 1) * P],
                                    ident[:NP, :NP])
            nc.scalar.copy(out=pt[:, :, :NP], in_=tr_ps[:, :, :NP])
            for c in range(FCH):
                nc.tensor.matmul(out_ps[:NP, :], lhsT=pt[:, c, :NP],
                                 rhs=wt_sb[:, c, :],
                                 start=(c == 0), stop=(c == FCH - 1))
            out_sb = outp.tile([P, FEAT], F32, tag="out_sb")
            nc.scalar.mul(out_sb[:NP, :], out_ps[:NP, :], denr[:NP, :])
            nc.sync.dma_start(out=out[goff:goff + NP, :], in_=out_sb[:NP, :])

        # ---- main loop: groups of (tile_start, n_batches); a small final
        # group keeps the exposed tail short. Tails pipelined one batch in ----
        GROUPS = [(0, 4), (32, 4), (64, 4), (96, 2), (112, 2)]
        prev = None
        for gi, (t0, nb) in enumerate(GROUPS):
            pool_ps = ps_pool.tile([P, FEAT], F32)
            den_ps = ps_den.tile([P, 1], F32)
            em_g = empool.tile([P, 32, W32], BF16, tag="em_g")
            last = gi == len(GROUPS) - 1
            for bu in range(nb):
                fin = last and bu == nb - 1
                nbatch = gi * 4 + bu if gi < 3 else 12 + (gi - 3) * 2 + bu
                emit_batch(t0, bu, pool_ps, den_ps, em_g,
                           splits=(0, 2, 4, 6) if fin else (0,),
                           offload=(nbatch >= 1 and not fin), extra_off=False)
                # previous group's tail: one batch of runway, except the
                # final group's predecessor (early, so the two closing
                # tails don't serialize after the last batch)
                if prev is not None and bu == (0 if last else 1):
                    emit_tail(*prev)
            prev = (t0 * GPT, nb, pool_ps, den_ps)
        emit_tail(*prev)
    nc.compile()
    return nc


def _host_inputs(atomwise_output, W, att_weight):
    """Per-core input maps (host prep: bf16 conversion + tiny mask tables)."""
    import ml_dtypes
    BF = ml_dtypes.bfloat16
    X = np.asarray(atomwise_output, dtype=np.float32)
    Xb = X.astype(BF)
    Wc = np.ascontiguousarray(np.asarray(W, dtype=np.float32))
    Wt = np.ascontiguousarray(Wc.T)
    att = np.asarray(att_weight, dtype=np.float32)
    v = Wt @ att                                               # v = W.T @ att
    Wtb = Wt.astype(BF)
    vrep = np.ascontiguousarray(np.broadcast_to(v, (P, FEAT))).astype(BF)
    # 0/1 mask tables (bf16 exact): mask01[p, k, c] = (c == 4k + p//32),
    # qmask[p, q] = (q == p//32)
    pp = np.arange(P)[:, None, None]
    kk = np.arange(8)[None, :, None]
    cc = np.arange(W32)[None, None, :]
    m01 = (cc == GPT * kk + pp // 32).astype(np.float32).astype(BF)
    m01 = np.ascontiguousarray(m01)
    qm = (np.arange(GPT)[None, :] == np.arange(P)[:, None] // 32)
    qm = np.ascontiguousarray(qm.astype(np.float32).astype(BF))
    in_maps = []
    for c in range(N_CORES):
        xc = Xb[c * NA_CORE:(c + 1) * NA_CORE]
        in_maps.append({"x": xc, "wt": Wtb, "vrep": vrep, "mask01": m01,
                        "qmask": qm})
    return in_maps


def _kernel_numpy_fallback(atomwise_output, n_atoms_i, W, att_weight):
    """Exact reference semantics in numpy (used only for non-uniform segments)."""
    X = np.asarray(atomwise_output, dtype=np.float32)
    n_at = np.asarray(n_atoms_i).astype(np.int64)
    W = np.asarray(W, dtype=np.float32)
    att = np.asarray(att_weight, dtype=np.float32)
    h = X @ W.T
    s = (att * h).sum(-1)
    s = np.where(s >= 0, s, NEG_SLOPE * s)
    seg = np.repeat(np.arange(len(n_at)), n_at)[:len(s)]
    ngr = len(n_at)
    smax = np.full(ngr, -np.inf, dtype=np.float32)
    np.maximum.at(smax, seg, s)
    e = np.exp(s - smax[seg])
    den = np.zeros(ngr, dtype=np.float32)
    np.add.at(den, seg, e)
    wgt = e / den[seg]
    outp = np.zeros((ngr, h.shape[1]), dtype=np.float32)
    np.add.at(outp, seg, wgt[:, None] * h)
    return outp


def _run_on_device(atomwise_output, W, att_weight):
    from concourse.bass_utils import run_bass_kernel_spmd

    if "nc" not in _CACHED:
        _CACHED["nc"] = _build_program()
    nc = _CACHED["nc"]
    in_maps = _host_inputs(atomwise_output, W, att_weight)
    res = run_bass_kernel_spmd(nc, in_maps, list(range(N_CORES)))
    return np.concatenate([res.results[c]["out"] for c in range(N_CORES)], axis=0)


def _run_in_subprocess(atomwise_output, n_atoms_i, W, att_weight):
    """Last-resort retry in a fresh process: a transient
    NRT_EXEC_UNIT_UNRECOVERABLE wedges the current NRT client session, but a
    new process (fresh axon boot) recovers. Arrays go via a temp dir."""
    import os, subprocess, sys, tempfile
    kdir = os.path.dirname(os.path.abspath(__file__))
    with tempfile.TemporaryDirectory() as td:
        np.save(os.path.join(td, "x.npy"), np.asarray(atomwise_output))
        np.save(os.path.join(td, "n.npy"), np.asarray(n_atoms_i))
        np.save(os.path.join(td, "w.npy"), np.asarray(W))
        np.save(os.path.join(td, "a.npy"), np.asarray(att_weight))
        driver = (
            "import sys, os, numpy as np\n"
            f"sys.path.insert(0, {kdir!r})\n"
            "import kernel\n"
            f"td = {td!r}\n"
            "out = kernel.kernel(np.load(td+'/x.npy'), np.load(td+'/n.npy'),\n"
            "                    np.load(td+'/w.npy'), np.load(td+'/a.npy'))\n"
            "np.save(td+'/out.npy', out)\n"
        )
        env = dict(os.environ, KERNEL_NO_SUBPROC="1")
        subprocess.run([sys.executable, "-c", driver], env=env, check=True,
                       timeout=1800)
        return np.load(os.path.join(td, "out.npy"))


def kernel(atomwise_output, n_atoms_i, W, att_weight):
    import os
    n_at = np.asarray(n_atoms_i)
    uniform = (
        atomwise_output.shape == (N_ATOMS, FEAT)
        and n_at.shape == (N_GRAPHS,)
        and np.all(n_at == N_ATOMS // N_GRAPHS)
    )
    if not uniform:
        return _kernel_numpy_fallback(atomwise_output, n_atoms_i, W, att_weight)

    try:
        out = _run_on_device(atomwise_output, W, att_weight)
    except Exception:
        try:
            out = _run_on_device(atomwise_output, W, att_weight)
        except Exception:
            if os.environ.get("KERNEL_NO_SUBPROC"):
                raise
            out = _run_in_subprocess(atomwise_output, n_atoms_i, W, att_weight)
    return out.astype(np.float32)
